# revision 19
# baseline (speedup 1.0000x reference)
"""MultiHeadAttention Trainium2 kernel (8 NeuronCores, Bass/Tile).

Problem: B=2, S=2048, D=1024, H=16, DK=64 fp32 MHA (torch-Linear style
projections, softmax attention, output projection).

Sharding: core c = (batch b = c//4, head-group g = c%4); each core handles
4 heads of one batch, entirely in a transposed layout (features on
partitions, sequence on the free axis):
  qhT/khT  = (W_g x^T + b)       [2 pairs x 128, 2048]
  vh       = x_v Wv_g^T          [2048, 4x65] (ones col -> row sums)
  scoresT  = khT^T qhT           per (pair, ktile, qtile) -> PSUM
  expT     = exp(scoresT/8)      ACT -> bf16
  rawT     = vh_aug^T expT       PV matmul; row 64 = softmax denominator
  outT     = rawT[0:64] * (1/rawT[64])
  partialT = woT^T outT          [1024, 2048] fp16 -> DRAM
Host: out[b] = sum_g partialT(b,g)^T + (Wo bv + bo).

v2 pipeline notes (v1 measured 257us, PE 75% busy):
- input DMA rings are serviced round-robin, so unordered loads all land
  at ~21us; ring chaining (chain_iter_dep) staggers wk->wq->xk->xq->xv
  so k-proj starts at ~7us.
- warmup matmuls ramp the PE out of its low p-state during the DMA wait.
- e2 exp tiles cycle through the same 32KB pool slots as the (dead by
  then) xk/xq/xv input tiles, giving 2 units of exp/PV pipelining
  without exceeding SBUF.
- partial output written fp16 (halves tail DMA); host sums in fp32.
"""

import numpy as np

B, S, D, H = 2, 2048, 1024, 16
DK = D // H          # 64
N_CORES = 8
HG = H // 4          # 4 head-groups
HL = 4               # heads per core
FEAT = HL * DK       # 256 per-core features
NQT = S // 512       # 4 query tiles
NKT = S // 128       # 16 key tiles
NDT = D // 128       # 8 contraction tiles (d-model)

DT_QK = "fp16"   # x_q/x_k, Wq/Wk, qhT/khT (score operands)
DT_V = "fp16"    # x_v, Wv
DT_PV = "bf16"   # vh_aug, expT
DT_O = "fp16"    # Wo, outT
N_WARMUP = 44    # PE p-state warmup matmuls during initial DMA wait

_cache = {}


def _np_dt(name):
    if name == "fp16":
        return np.float16
    import ml_dtypes
    return ml_dtypes.bfloat16


def _build():
    import concourse.mybir as mybir
    import concourse.tile as tile
    from concourse import bacc

    fp32 = mybir.dt.float32
    dt_qk = getattr(mybir.dt, "float16" if DT_QK == "fp16" else "bfloat16")
    dt_v = getattr(mybir.dt, "float16" if DT_V == "fp16" else "bfloat16")
    dt_pv = getattr(mybir.dt, "float16" if DT_PV == "fp16" else "bfloat16")
    dt_o = getattr(mybir.dt, "float16" if DT_O == "fp16" else "bfloat16")
    dt_out = mybir.dt.float16

    nc = bacc.Bacc("TRN2", target_bir_lowering=False, debug=False,
                   num_devices=N_CORES)

    # all inputs host-swizzled to [128, ...] so each SBUF partition row is
    # ONE contiguous DRAM read. DMA queues round-robin per DESCRIPTOR, so
    # descriptor size acts as priority: x uses 32KB descriptors, weights
    # ~8KB. qk biases ride in the qk weight buffer (fp32 bit-packed into 4
    # trailing fp16 columns, bitcast on device) to avoid tiny descriptors.
    xqT = nc.dram_tensor("xqT", [128, NDT * S], dt_qk,
                         kind="ExternalInput").ap()
    xkT = nc.dram_tensor("xkT", [128, NDT * S], dt_qk,
                         kind="ExternalInput").ap()
    xvT = nc.dram_tensor("xvT", [128, NDT * S], dt_v,
                         kind="ExternalInput").ap()
    wkbT = nc.dram_tensor("wkbT", [128, NDT * FEAT + 4], dt_qk,
                          kind="ExternalInput").ap()
    wqbT = nc.dram_tensor("wqbT", [128, NDT * FEAT + 4], dt_qk,
                          kind="ExternalInput").ap()
    wvoT = nc.dram_tensor("wvoT", [128, NDT * FEAT + 2 * D], dt_v,
                          kind="ExternalInput").ap()
    # output layout [qt, p, jt, s]: each partition row is one contiguous
    # 8KB write (fewer, bigger DMA descriptors); host re-transposes
    out_d = nc.dram_tensor("partialT", [NQT, 128, NDT, 512], dt_out,
                           kind="ExternalOutput").ap()

    xq_r = xqT.rearrange("p (t s) -> p t s", t=NDT)
    xk_r = xkT.rearrange("p (t s) -> p t s", t=NDT)
    xv_r = xvT.rearrange("p (t s) -> p t s", t=NDT)

    with tile.TileContext(nc) as tc:
        def chain(inst, key):
            # stagger DMA ring groups: rings within a group run in parallel
            # (full HBM bw); later groups start only after the prior group's
            # lane finishes, so early tensors land first.
            try:
                tc.chain_iter_dep(key, inst)
            except Exception:
                pass

        with (
            tc.tile_pool(name="win", bufs=1) as win,
            tc.tile_pool(name="big", bufs=4) as big,
            tc.tile_pool(name="proj", bufs=1) as proj,
            tc.tile_pool(name="pout", bufs=1) as pout,
            tc.tile_pool(name="pnrm", bufs=2) as pnrm,
            tc.tile_pool(name="pp", bufs=2, space="PSUM") as pp,
            tc.tile_pool(name="ps2", bufs=2, space="PSUM") as ps2,
            tc.tile_pool(name="pspv", bufs=2, space="PSUM") as pspv,
        ):
            wdum0 = win.tile([128, 512], dt_qk, tag="wdum")
            junk = win.tile([128, 512], dt_qk, tag="junk")
            nc.vector.memset(wdum0[:], 0.0)
            # ---- DMA: one big-descriptor ring per tensor; 3-hop chain
            # xk -> xq -> xv -> wvo so earlier-needed tensors get the full
            # link. wqk (weights+biases) rides unchained beside xk. ----
            wkb = win.tile([128, NDT * FEAT + 4], dt_qk, tag="wkb")
            wqb = win.tile([128, NDT * FEAT + 4], dt_qk, tag="wqb")
            wvo = win.tile([128, NDT * FEAT + 2 * D], dt_v, tag="wvo")

            xk3 = big.tile([128, NDT, S], dt_qk, tag="big")
            xq3 = big.tile([128, NDT, S], dt_qk, tag="big")
            xv3 = big.tile([128, NDT, S], dt_v, tag="big")

            chain(nc.sync.dma_start(wkb[:], wkbT), "l0")
            nc.scalar.activation(junk[0:1, 0:1], wdum0[0:1, 0:1],
                                 mybir.ActivationFunctionType.Exp, scale=1.0)
            for t0 in (0, 4):
                chain(nc.sync.dma_start(xk3[:, t0:t0 + 4, :],
                                        xk_r[:, t0:t0 + 4, :]), "l0")
            chain(nc.sync.dma_start(wqb[:], wqbT), "l0")
            for x3, xr in ((xq3, xq_r), (xv3, xv_r)):
                for t0 in (0, 4):
                    chain(nc.sync.dma_start(x3[:, t0:t0 + 4, :],
                                            xr[:, t0:t0 + 4, :]), "l0")
            chain(nc.sync.dma_start(wvo[:], wvoT), "l0")

            wk3 = wkb[:, 0:NDT * FEAT].rearrange("p (t f) -> p t f", t=NDT)
            wq3 = wqb[:, 0:NDT * FEAT].rearrange("p (t f) -> p t f", t=NDT)
            bk3 = wkb[:, NDT * FEAT:NDT * FEAT + 4].bitcast(fp32)
            bq3 = wqb[:, NDT * FEAT:NDT * FEAT + 4].bitcast(fp32)
            wv3 = wvo[:, 0:NDT * FEAT].rearrange("p (t f) -> p t f", t=NDT)
            wo3 = wvo[:, NDT * FEAT:].rearrange("p (t j) -> p t j", t=2)

            # ---- persistent intermediates ----
            qh3 = proj.tile([128, 2, S], dt_qk, tag="qh")   # pair-packed
            kh3 = proj.tile([128, 2, S], dt_qk, tag="kh")
            vha = proj.tile([128, NKT, HL, DK + 1], dt_pv, tag="vha")
            ot3 = proj.tile([128, 2, S], dt_o, tag="outT")
            nc.gpsimd.memset(vha[:, :, :, DK], 1.0)  # ones col -> denominators

            # ---- PE p-state warmup while the first DMAs land ----
            wdum = wdum0
            wu = pp.tile([128, 512], fp32, tag="acc")
            for i in range(N_WARMUP):
                nc.tensor.matmul(wu[:], wdum[:, 0:128], wdum[:],
                                 start=(i == 0), stop=(i == N_WARMUP - 1))
            nc.vector.tensor_copy(junk[:], wu[:])

            # ---- projections: 2 psum accumulators per pass, kt-interleaved
            # so matmuls chase the chunked x DMAs ----
            def qk_pass(x3, w3, b3, dst, m, nn):
                accs = [pp.tile([128, 512], fp32, tag="acc", name=f"acc{n}")
                        for n in nn]
                for kt in range(NDT):
                    for a, n in zip(accs, nn):
                        nc.tensor.matmul(
                            a[:], w3[:, kt, m * 128:(m + 1) * 128],
                            x3[:, kt, n * 512:(n + 1) * 512],
                            start=(kt == 0), stop=(kt == NDT - 1))
                for a, n in zip(accs, nn):
                    nc.vector.tensor_scalar_add(
                        dst[:, m, n * 512:(n + 1) * 512], a[:],
                        b3[:, m:m + 1])

            def v_proj():
                for st in range(NKT):
                    ps = pp.tile([128, 512], fp32, tag="acc")
                    for kt in range(NDT):
                        nc.tensor.matmul(
                            ps[:, 0:256], xv3[:, kt, st * 128:(st + 1) * 128],
                            wv3[:, kt, :],
                            start=(kt == 0), stop=(kt == NDT - 1))
                    nc.vector.tensor_copy(vha[:, st, :, 0:DK],
                                          ps[:, 0:256])

            def attn_scores(qt, hp, e2u):
                for kt in range(NKT):
                    s2 = ps2.tile([128, 1024], fp32, tag="s2")
                    nc.tensor.matmul(
                        s2[:, 0:512],
                        kh3[0:64, hp, kt * 128:(kt + 1) * 128],
                        qh3[0:64, hp, qt * 512:(qt + 1) * 512],
                        start=True, stop=True)
                    nc.tensor.matmul(
                        s2[:, 512:1024],
                        kh3[64:128, hp, kt * 128:(kt + 1) * 128],
                        qh3[64:128, hp, qt * 512:(qt + 1) * 512],
                        start=True, stop=True)
                    nc.scalar.activation(
                        e2u[:, kt, :], s2[:],
                        mybir.ActivationFunctionType.Exp, scale=0.125)

            def attn_pv(qt, hp, e2u):
                pva = pspv.tile([DK + 1, 512], fp32, tag="pv")
                pvb = pspv.tile([DK + 1, 512], fp32, tag="pv")
                for kt in range(NKT):
                    nc.tensor.matmul(
                        pva[:], vha[:, kt, 2 * hp, :], e2u[:, kt, 0:512],
                        start=(kt == 0), stop=(kt == NKT - 1))
                    nc.tensor.matmul(
                        pvb[:], vha[:, kt, 2 * hp + 1, :],
                        e2u[:, kt, 512:1024],
                        start=(kt == 0), stop=(kt == NKT - 1))
                for pv, half in ((pva, 0), (pvb, 1)):
                    # copy the whole accumulator to SBUF first: frees the
                    # PSUM bank for the next unit's PV in ~0.7us instead of
                    # after the full normalize chain; DVE cost is the same
                    # (free-size bound) and custom DVE ops need SBUF anyway.
                    pvs = pnrm.tile([DK + 1, 512], fp32, tag="pvs")
                    nc.vector.tensor_copy(pvs[:], pv[:])
                    # custom-DVE recip needs a base-partition-0 input; BIR
                    # also requires partition starts in {0,32,64,96}, so the
                    # denominator row (partition 64) is staged via srow
                    srow = pnrm.tile([1, 512], fp32, tag="srow")
                    nc.vector.tensor_copy(srow[:], pvs[DK:DK + 1, :])
                    inv = pnrm.tile([1, 512], fp32, tag="inv")
                    nc.vector.reciprocal_approx_fast(inv[:], srow[:])
                    invb = pnrm.tile([64, 512], fp32, tag="invb")
                    nc.gpsimd.partition_broadcast(invb[:], inv[:])
                    nc.vector.tensor_tensor(
                        ot3[half * 64:(half + 1) * 64, hp,
                            qt * 512:(qt + 1) * 512],
                        pvs[0:DK, :], invb[:], mybir.AluOpType.mult)

            def oproj(qt):
                po = pout.tile([128, NDT, 512], dt_out, tag="po", bufs=1)
                for jt in range(NDT):
                    ps = pp.tile([128, 512], fp32, tag="acc")
                    for m in range(2):
                        nc.tensor.matmul(
                            ps[:], wo3[:, m, jt * 128:(jt + 1) * 128],
                            ot3[:, m, qt * 512:(qt + 1) * 512],
                            start=(m == 0), stop=(m == 1))
                    if jt % 2 == 0:
                        nc.vector.tensor_copy(po[:, jt, :], ps[:])
                    else:
                        nc.scalar.copy(po[:, jt, :], ps[:])
                nc.sync.dma_start(out_d[qt], po[:])

            def e2tile(name):
                return big.tile([128, NKT, 1024], dt_pv, tag="big", name=name)

            # ---- emission order == per-engine execution order ----
            for m in range(2):                       # k-proj (all pairs)
                for nn in ((0, 1), (2, 3)):
                    qk_pass(xk3, wk3, bk3, kh3, m, nn)
            qk_pass(xq3, wq3, bq3, qh3, 0, (0,))     # q-proj heads only
            qk_pass(xq3, wq3, bq3, qh3, 1, (0,))
            e00 = e2tile("e00")
            attn_scores(0, 0, e00)                   # ACT starts here
            e01 = e2tile("e01")
            attn_scores(0, 1, e01)
            for m in range(2):                       # q-proj remainder
                qk_pass(xq3, wq3, bq3, qh3, m, (1, 2))
                qk_pass(xq3, wq3, bq3, qh3, m, (3,))
            v_proj()
            attn_pv(0, 0, e00)
            e10 = e2tile("e10")
            attn_scores(1, 0, e10)
            attn_pv(0, 1, e01)
            oproj(0)
            prev = {(1, 0): e10}
            for qt in range(1, NQT):
                e_b = e2tile(f"e{qt}1")
                attn_scores(qt, 1, e_b)
                attn_pv(qt, 0, prev[(qt, 0)])
                if qt < NQT - 1:
                    e_a = e2tile(f"e{qt + 1}0")
                    attn_scores(qt + 1, 0, e_a)
                    prev[(qt + 1, 0)] = e_a
                attn_pv(qt, 1, e_b)
                if qt == NQT - 1:
                    wu2 = pp.tile([128, 512], fp32, tag="acc", name="wu2")
                    for i in range(10):
                        nc.tensor.matmul(wu2[:], wdum[:, 0:128], wdum[:],
                                         start=(i == 0), stop=(i == 9))
                    nc.vector.tensor_copy(junk[:], wu2[:])
                oproj(qt)

    nc.compile()
    return nc


def kernel(q, k, v, Wq, bq, Wk, bk, Wv, bv, Wo, bo, _trace=False):
    from concourse import bass_utils

    if "nc" not in _cache:
        _cache["nc"] = _build()
    nc = _cache["nc"]

    q = np.asarray(q, np.float32)
    k = np.asarray(k, np.float32)
    v = np.asarray(v, np.float32)
    Wq = np.asarray(Wq, np.float32)
    Wk = np.asarray(Wk, np.float32)
    Wv = np.asarray(Wv, np.float32)
    Wo = np.asarray(Wo, np.float32)
    bq = np.asarray(bq, np.float32)
    bk = np.asarray(bk, np.float32)
    bv = np.asarray(bv, np.float32)
    bo = np.asarray(bo, np.float32)

    d_qk, d_v, d_o = _np_dt(DT_QK), _np_dt(DT_V), _np_dt(DT_O)

    def swz(a, t):     # [t*128, f] -> [128, t*f], rows contiguous in DRAM
        f = a.shape[1]
        return np.ascontiguousarray(
            a.reshape(t, 128, f).transpose(1, 0, 2).reshape(128, t * f))

    xT = {}
    for b in range(B):
        xT[("q", b)] = swz(np.ascontiguousarray(q[b].T), NDT).astype(d_qk)
        xT[("k", b)] = swz(np.ascontiguousarray(k[b].T), NDT).astype(d_qk)
        xT[("v", b)] = swz(np.ascontiguousarray(v[b].T), NDT).astype(d_v)
    wT = {}
    for g in range(HG):
        sl = slice(g * FEAT, (g + 1) * FEAT)
        wk_s = swz(np.ascontiguousarray(Wk[sl, :].T), NDT).astype(d_qk)
        wq_s = swz(np.ascontiguousarray(Wq[sl, :].T), NDT).astype(d_qk)
        bk_s = np.ascontiguousarray(
            bk[sl].astype(np.float32).reshape(2, 128).T).view(np.uint16)
        bq_s = np.ascontiguousarray(
            bq[sl].astype(np.float32).reshape(2, 128).T).view(np.uint16)
        wv_s = swz(np.ascontiguousarray(Wv[sl, :].T), NDT).astype(d_v)
        wo_s = swz(np.ascontiguousarray(Wo[:, sl].T), 2).astype(d_o)
        wT[("kb", g)] = np.ascontiguousarray(np.concatenate(
            [wk_s.view(np.uint16), bk_s], axis=1).view(d_qk))
        wT[("qb", g)] = np.ascontiguousarray(np.concatenate(
            [wq_s.view(np.uint16), bq_s], axis=1).view(d_qk))
        wT[("vo", g)] = np.ascontiguousarray(
            np.concatenate([wv_s, wo_s], axis=1))

    in_maps = []
    for c in range(N_CORES):
        b, g = divmod(c, HG)
        in_maps.append({
            "xqT": xT[("q", b)], "xkT": xT[("k", b)], "xvT": xT[("v", b)],
            "wkbT": wT[("kb", g)], "wqbT": wT[("qb", g)],
            "wvoT": wT[("vo", g)],
        })

    kwargs = {}
    if _trace:
        _install_profile_shim()
        kwargs = dict(trace=True, trace_cores=list(range(N_CORES)))
    res = bass_utils.run_bass_kernel_spmd(
        nc, in_maps, core_ids=list(range(N_CORES)), **kwargs)
    _cache["last_results"] = res

    final_bias = (Wo @ bv + bo).astype(np.float32)  # attn rows sum to 1
    out = np.empty((B, S, D), np.float32)
    for b in range(B):
        acc = res.results[b * HG]["partialT"].astype(np.float32)
        for g in range(1, HG):
            acc += res.results[b * HG + g]["partialT"].astype(np.float32)
        # [qt, p, jt, s] -> [S, D]:  d = jt*128+p, q = qt*512+s
        out[b] = acc.transpose(0, 3, 2, 1).reshape(S, D) + final_bias
    return out


def _install_profile_shim():
    """Provide antenv.axon_hooks so trace=True works under axon."""
    import sys
    import types

    import antenv

    if "antenv.axon_hooks" in sys.modules:
        return
    mod = types.ModuleType("antenv.axon_hooks")
    mod._hook = None
    mod.set_axon_ntff_profile_hook = lambda h: setattr(mod, "_hook", h)
    mod.get_axon_ntff_profile_hook = lambda: mod._hook
    sys.modules["antenv.axon_hooks"] = mod
    antenv.axon_hooks = mod
    try:
        from trn_agent_boot.trn_boot import _ntff_profile_via_ctypes
        mod.set_axon_ntff_profile_hook(
            _ntff_profile_via_ctypes("/opt/axon/libaxon_pjrt.so"))
    except Exception:
        pass


# revision 21
# speedup vs baseline: 1.0649x; 1.0649x over previous
"""MultiHeadAttention Trainium2 kernel (8 NeuronCores, Bass/Tile).

Problem: B=2, S=2048, D=1024, H=16, DK=64 fp32 MHA (torch-Linear style
projections, softmax attention, output projection).

Sharding: core c = (batch b = c//4, head-group g = c%4); each core handles
4 heads of one batch, entirely in a transposed layout (features on
partitions, sequence on the free axis):
  qhT/khT  = (W_g x^T + b)       [2 pairs x 128, 2048]
  vh       = x_v Wv_g^T          [2048, 4x65] (ones col -> row sums)
  scoresT  = khT^T qhT           per (pair, ktile, qtile) -> PSUM
  expT     = exp(scoresT/8)      ACT -> bf16
  rawT     = vh_aug^T expT       PV matmul; row 64 = softmax denominator
  outT     = rawT[0:64] * (1/rawT[64])
  partialT = woT^T outT          [1024, 2048] fp16 -> DRAM
Host: out[b] = sum_g partialT(b,g)^T + (Wo bv + bo).

v2 pipeline notes (v1 measured 257us, PE 75% busy):
- input DMA rings are serviced round-robin, so unordered loads all land
  at ~21us; ring chaining (chain_iter_dep) staggers wk->wq->xk->xq->xv
  so k-proj starts at ~7us.
- warmup matmuls ramp the PE out of its low p-state during the DMA wait.
- e2 exp tiles cycle through the same 32KB pool slots as the (dead by
  then) xk/xq/xv input tiles, giving 2 units of exp/PV pipelining
  without exceeding SBUF.
- partial output written fp16 (halves tail DMA); host sums in fp32.
"""

import numpy as np

B, S, D, H = 2, 2048, 1024, 16
DK = D // H          # 64
N_CORES = 8
HG = H // 4          # 4 head-groups
HL = 4               # heads per core
FEAT = HL * DK       # 256 per-core features
NQT = S // 512       # 4 query tiles
NKT = S // 128       # 16 key tiles
NDT = D // 128       # 8 contraction tiles (d-model)

DT_QK = "fp16"   # x_q/x_k, Wq/Wk, qhT/khT (score operands)
DT_V = "fp16"    # x_v, Wv
DT_PV = "bf16"   # vh_aug, expT
DT_O = "fp16"    # Wo, outT
N_WARMUP = 44    # PE p-state warmup matmuls during initial DMA wait

_cache = {}


def _np_dt(name):
    if name == "fp16":
        return np.float16
    import ml_dtypes
    return ml_dtypes.bfloat16


def _build():
    import concourse.mybir as mybir
    import concourse.tile as tile
    from concourse import bacc

    fp32 = mybir.dt.float32
    dt_qk = getattr(mybir.dt, "float16" if DT_QK == "fp16" else "bfloat16")
    dt_v = getattr(mybir.dt, "float16" if DT_V == "fp16" else "bfloat16")
    dt_pv = getattr(mybir.dt, "float16" if DT_PV == "fp16" else "bfloat16")
    dt_o = getattr(mybir.dt, "float16" if DT_O == "fp16" else "bfloat16")
    dt_out = mybir.dt.float16

    nc = bacc.Bacc("TRN2", target_bir_lowering=False, debug=False,
                   num_devices=N_CORES)

    # all inputs host-swizzled to [128, ...] so each SBUF partition row is
    # ONE contiguous DRAM read. DMA queues round-robin per DESCRIPTOR, so
    # descriptor size acts as priority: x uses 32KB descriptors, weights
    # ~8KB. qk biases ride in the qk weight buffer (fp32 bit-packed into 4
    # trailing fp16 columns, bitcast on device) to avoid tiny descriptors.
    xqT = nc.dram_tensor("xqT", [128, NDT * S], dt_qk,
                         kind="ExternalInput").ap()
    xkT = nc.dram_tensor("xkT", [128, NDT * S], dt_qk,
                         kind="ExternalInput").ap()
    xvT = nc.dram_tensor("xvT", [128, NDT * S], dt_v,
                         kind="ExternalInput").ap()
    wkbT = nc.dram_tensor("wkbT", [128, NDT * FEAT + 4], dt_qk,
                          kind="ExternalInput").ap()
    wqbT = nc.dram_tensor("wqbT", [128, NDT * FEAT + 4], dt_qk,
                          kind="ExternalInput").ap()
    wvoT = nc.dram_tensor("wvoT", [128, NDT * FEAT + 2 * D], dt_v,
                          kind="ExternalInput").ap()
    # output layout [qt, p, jt, s]: each partition row is one contiguous
    # 8KB write (fewer, bigger DMA descriptors); host re-transposes
    out_d = nc.dram_tensor("partialT", [NQT, 128, NDT, 512], dt_out,
                           kind="ExternalOutput").ap()

    xq_r = xqT.rearrange("p (t s) -> p t s", t=NDT)
    xk_r = xkT.rearrange("p (t s) -> p t s", t=NDT)
    xv_r = xvT.rearrange("p (t s) -> p t s", t=NDT)

    with tile.TileContext(nc) as tc:
        def chain(inst, key):
            # stagger DMA ring groups: rings within a group run in parallel
            # (full HBM bw); later groups start only after the prior group's
            # lane finishes, so early tensors land first.
            try:
                tc.chain_iter_dep(key, inst)
            except Exception:
                pass

        with (
            tc.tile_pool(name="win", bufs=1) as win,
            tc.tile_pool(name="big", bufs=4) as big,
            tc.tile_pool(name="proj", bufs=1) as proj,
            tc.tile_pool(name="pout", bufs=1) as pout,
            tc.tile_pool(name="pnrm", bufs=2) as pnrm,
            tc.tile_pool(name="pp", bufs=2, space="PSUM") as pp,
            tc.tile_pool(name="ps2", bufs=2, space="PSUM") as ps2,
            tc.tile_pool(name="pspv", bufs=2, space="PSUM") as pspv,
        ):
            wdum0 = win.tile([128, 512], dt_qk, tag="wdum")
            junk = win.tile([128, 512], dt_qk, tag="junk")
            nc.vector.memset(wdum0[:], 0.0)
            # ---- DMA: one big-descriptor ring per tensor; 3-hop chain
            # xk -> xq -> xv -> wvo so earlier-needed tensors get the full
            # link. wqk (weights+biases) rides unchained beside xk. ----
            wkb = win.tile([128, NDT * FEAT + 4], dt_qk, tag="wkb")
            wqb = win.tile([128, NDT * FEAT + 4], dt_qk, tag="wqb")
            wvo = win.tile([128, NDT * FEAT + 2 * D], dt_v, tag="wvo")

            xk3 = big.tile([128, NDT, S], dt_qk, tag="big")
            xq3 = big.tile([128, NDT, S], dt_qk, tag="big")
            xv3 = big.tile([128, NDT, S], dt_v, tag="big")

            chain(nc.sync.dma_start(wkb[:], wkbT), "l0")
            nc.scalar.activation(junk[0:1, 0:1], wdum0[0:1, 0:1],
                                 mybir.ActivationFunctionType.Exp, scale=1.0)
            for t0 in (0, 4):
                chain(nc.sync.dma_start(xk3[:, t0:t0 + 4, :],
                                        xk_r[:, t0:t0 + 4, :]), "l0")
            chain(nc.sync.dma_start(wqb[:], wqbT), "l0")
            for x3, xr in ((xq3, xq_r), (xv3, xv_r)):
                for t0 in (0, 4):
                    chain(nc.sync.dma_start(x3[:, t0:t0 + 4, :],
                                            xr[:, t0:t0 + 4, :]), "l0")
            chain(nc.sync.dma_start(wvo[:], wvoT), "l0")

            wk3 = wkb[:, 0:NDT * FEAT].rearrange("p (t f) -> p t f", t=NDT)
            wq3 = wqb[:, 0:NDT * FEAT].rearrange("p (t f) -> p t f", t=NDT)
            bk3 = wkb[:, NDT * FEAT:NDT * FEAT + 4].bitcast(fp32)
            bq3 = wqb[:, NDT * FEAT:NDT * FEAT + 4].bitcast(fp32)
            wv3 = wvo[:, 0:NDT * FEAT].rearrange("p (t f) -> p t f", t=NDT)
            wo3 = wvo[:, NDT * FEAT:].rearrange("p (t j) -> p t j", t=2)

            # ---- persistent intermediates ----
            qh3 = proj.tile([128, 2, S], dt_qk, tag="qh")   # pair-packed
            kh3 = proj.tile([128, 2, S], dt_qk, tag="kh")
            vha = proj.tile([128, NKT, HL, DK + 1], dt_pv, tag="vha")
            ot3 = proj.tile([128, 2, S], dt_o, tag="outT")
            nc.gpsimd.memset(vha[:, :, :, DK], 1.0)  # ones col -> denominators

            # ---- PE p-state warmup while the first DMAs land ----
            wdum = wdum0
            wu = pp.tile([128, 512], fp32, tag="acc")
            for i in range(N_WARMUP):
                nc.tensor.matmul(wu[:], wdum[:, 0:128], wdum[:],
                                 start=(i == 0), stop=(i == N_WARMUP - 1))
            nc.vector.tensor_copy(junk[:], wu[:])

            # ---- projections: 2 psum accumulators per pass, kt-interleaved
            # so matmuls chase the chunked x DMAs ----
            def qk_pass(x3, w3, b3, dst, m, nn):
                accs = [pp.tile([128, 512], fp32, tag="acc", name=f"acc{n}")
                        for n in nn]
                for kt in range(NDT):
                    for a, n in zip(accs, nn):
                        nc.tensor.matmul(
                            a[:], w3[:, kt, m * 128:(m + 1) * 128],
                            x3[:, kt, n * 512:(n + 1) * 512],
                            start=(kt == 0), stop=(kt == NDT - 1))
                for a, n in zip(accs, nn):
                    nc.vector.tensor_scalar_add(
                        dst[:, m, n * 512:(n + 1) * 512], a[:],
                        b3[:, m:m + 1])

            def v_quantum(st):
                ps = pp.tile([128, 512], fp32, tag="acc", name="vacc")
                for kt in range(NDT):
                    nc.tensor.matmul(
                        ps[:, 0:256], xv3[:, kt, st * 128:(st + 1) * 128],
                        wv3[:, kt, :],
                        start=(kt == 0), stop=(kt == NDT - 1))
                nc.vector.tensor_copy(vha[:, st, :, 0:DK], ps[:, 0:256])

            def pv_quantum(state, qt, hp, e2u, kt):
                if "a" not in state:
                    state["a"] = pspv.tile([DK + 1, 512], fp32, tag="pv",
                                           name="pva")
                    state["b"] = pspv.tile([DK + 1, 512], fp32, tag="pv",
                                           name="pvb")
                nc.tensor.matmul(
                    state["a"][:], vha[:, kt, 2 * hp, :], e2u[:, kt, 0:512],
                    start=(kt == 0), stop=(kt == NKT - 1))
                nc.tensor.matmul(
                    state["b"][:], vha[:, kt, 2 * hp + 1, :],
                    e2u[:, kt, 512:1024],
                    start=(kt == 0), stop=(kt == NKT - 1))

            def norm(state, qt, hp):
                # whole-accumulator copy frees the PSUM bank early; custom
                # DVE recip needs a base-partition-0 SBUF input (srow)
                for pv, half in ((state["a"], 0), (state["b"], 1)):
                    pvs = pnrm.tile([DK + 1, 512], fp32, tag="pvs")
                    nc.vector.tensor_copy(pvs[:], pv[:])
                    srow = pnrm.tile([1, 512], fp32, tag="srow")
                    nc.vector.tensor_copy(srow[:], pvs[DK:DK + 1, :])
                    inv = pnrm.tile([1, 512], fp32, tag="inv")
                    nc.vector.reciprocal_approx_fast(inv[:], srow[:])
                    invb = pnrm.tile([64, 512], fp32, tag="invb")
                    nc.gpsimd.partition_broadcast(invb[:], inv[:])
                    nc.vector.tensor_tensor(
                        ot3[half * 64:(half + 1) * 64, hp,
                            qt * 512:(qt + 1) * 512],
                        pvs[0:DK, :], invb[:], mybir.AluOpType.mult)

            def oproj_quantum(pstate, qt, jt):
                if "po" not in pstate:
                    pstate["po"] = pout.tile([128, NDT, 512], dt_out,
                                             tag="po", bufs=1, name="po")
                ps = pp.tile([128, 512], fp32, tag="acc", name="oacc")
                for m in range(2):
                    nc.tensor.matmul(
                        ps[:], wo3[:, m, jt * 128:(jt + 1) * 128],
                        ot3[:, m, qt * 512:(qt + 1) * 512],
                        start=(m == 0), stop=(m == 1))
                nc.vector.tensor_copy(pstate["po"][:, jt, :], ps[:])
                if jt == NDT - 1:
                    nc.sync.dma_start(out_d[qt], pstate["po"][:])

            def e2tile(name):
                return big.tile([128, NKT, 1024], dt_pv, tag="big", name=name)

            # ---- filler queue: PE work that rides in the slack of the
            # exp-paced scores stream (ACT needs ~1088ns/kt, scores only
            # ~426ns of PE) so the exp stream never starves ----
            fillers = []      # list of (cost_ns, thunk)
            fq = {"i": 0, "budget": 0.0}

            def drain_fillers():
                while fq["i"] < len(fillers):
                    fillers[fq["i"]][1]()
                    fq["i"] += 1

            SLACK_NS = 620    # filler budget added per exp-paced kt step

            def s_unit(qt, hp, e2u, fill=True):
                for kt in range(NKT):
                    s2 = ps2.tile([128, 1024], fp32, tag="s2")
                    nc.tensor.matmul(
                        s2[:, 0:512],
                        kh3[0:64, hp, kt * 128:(kt + 1) * 128],
                        qh3[0:64, hp, qt * 512:(qt + 1) * 512],
                        start=True, stop=True)
                    nc.tensor.matmul(
                        s2[:, 512:1024],
                        kh3[64:128, hp, kt * 128:(kt + 1) * 128],
                        qh3[64:128, hp, qt * 512:(qt + 1) * 512],
                        start=True, stop=True)
                    nc.scalar.activation(
                        e2u[:, kt, :], s2[:],
                        mybir.ActivationFunctionType.Exp, scale=0.125)
                    if not fill:
                        continue
                    fq["budget"] += SLACK_NS
                    while (fq["i"] < len(fillers)
                           and fillers[fq["i"]][0] <= fq["budget"]):
                        cost, fn = fillers[fq["i"]]
                        fq["budget"] -= cost
                        fn()
                        fq["i"] += 1

            # ---- emission: K, q-heads, then the exp-paced score stream
            # with everything else as fillers. Queue order respects the
            # e2-slot reuse deps (all xq readers before S(1,0)'s exps emit,
            # all xv readers before S(1,1)'s). ----
            for m in range(2):                       # k-proj (all pairs)
                for nn in ((0, 1), (2, 3)):
                    qk_pass(xk3, wk3, bk3, kh3, m, nn)
            qk_pass(xq3, wq3, bq3, qh3, 0, (0,))     # q-proj heads only
            qk_pass(xq3, wq3, bq3, qh3, 1, (0,))

            for m in range(2):                       # q remainder -> fillers
                for n in (1, 2, 3):
                    fillers.append(
                        (1800, lambda m=m, n=n: qk_pass(
                            xq3, wq3, bq3, qh3, m, (n,))))

            e00 = e2tile("e00")
            s_unit(0, 0, e00, fill=False)            # ACT starts here
            e01 = e2tile("e01")
            s_unit(0, 1, e01)
            # all xq readers are queued; force them out before S(1,0) emits
            # exp writes into xq3's recycled slot
            drain_fillers()
            for st in range(NKT):                    # v-proj -> fillers
                fillers.append((900, lambda st=st: v_quantum(st)))
            e10 = e2tile("e10")
            s_unit(1, 0, e10)
            drain_fillers()                          # xv readers, see above
            pv_states = {}
            for u, (uq, uh, eu) in enumerate(((0, 0, e00), (0, 1, e01))):
                st_ = pv_states[(uq, uh)] = {}
                for kt in range(NKT):
                    fillers.append(
                        (440, lambda s=st_, q=uq, h=uh, e=eu, k=kt:
                         pv_quantum(s, q, h, e, k)))
                fillers.append(
                    (0, lambda s=st_, q=uq, h=uh: norm(s, q, h)))
            e11 = e2tile("e11")
            s_unit(1, 1, e11)

            prev = {(1, 0): e10, (1, 1): e11}
            for qt in range(1, NQT):
                for hp in range(2):
                    if qt == NQT - 1 and hp == 1:
                        continue          # last unit's PV runs in the tail
                    st_ = pv_states[(qt, hp)] = {}
                    eu = prev[(qt, hp)]
                    for kt in range(NKT):
                        fillers.append(
                            (440, lambda s=st_, q=qt, h=hp, e=eu, k=kt:
                             pv_quantum(s, q, h, e, k)))
                    fillers.append(
                        (0, lambda s=st_, q=qt, h=hp: norm(s, q, h)))
                    if hp == 1:
                        pstate = {}
                        for jt in range(NDT):
                            fillers.append(
                                (480, lambda p=pstate, q=qt - 1, j=jt:
                                 oproj_quantum(p, q, j)))
                if qt == NQT - 1:
                        pstate = {}
                        for jt in range(NDT):
                            fillers.append(
                                (480, lambda p=pstate, q=qt - 1, j=jt:
                                 oproj_quantum(p, q, j)))
                # emit next qt's score units
                if qt < NQT - 1:
                    for hp in range(2):
                        e_n = e2tile(f"e{qt + 1}{hp}")
                        prev[(qt + 1, hp)] = e_n
                        s_unit(qt + 1, hp, e_n)

            # tail: leftover fillers, last PV tracking the last exps, then
            # keeper matmuls bridge the final normalize so oproj(3) stays at
            # full clock
            drain_fillers()
            st_ = pv_states[(3, 1)] = {}
            for kt in range(NKT):
                pv_quantum(st_, 3, 1, prev[(3, 1)], kt)
            wu2 = pp.tile([128, 512], fp32, tag="acc", name="wu2")
            for i in range(10):
                nc.tensor.matmul(wu2[:], wdum[:, 0:128], wdum[:],
                                 start=(i == 0), stop=(i == 9))
            norm(st_, 3, 1)
            nc.vector.tensor_copy(junk[:], wu2[:])
            pstate = {}
            for jt in range(NDT):
                oproj_quantum(pstate, 3, jt)

    nc.compile()
    return nc


def kernel(q, k, v, Wq, bq, Wk, bk, Wv, bv, Wo, bo, _trace=False):
    from concourse import bass_utils

    if "nc" not in _cache:
        _cache["nc"] = _build()
    nc = _cache["nc"]

    q = np.asarray(q, np.float32)
    k = np.asarray(k, np.float32)
    v = np.asarray(v, np.float32)
    Wq = np.asarray(Wq, np.float32)
    Wk = np.asarray(Wk, np.float32)
    Wv = np.asarray(Wv, np.float32)
    Wo = np.asarray(Wo, np.float32)
    bq = np.asarray(bq, np.float32)
    bk = np.asarray(bk, np.float32)
    bv = np.asarray(bv, np.float32)
    bo = np.asarray(bo, np.float32)

    d_qk, d_v, d_o = _np_dt(DT_QK), _np_dt(DT_V), _np_dt(DT_O)

    def swz(a, t):     # [t*128, f] -> [128, t*f], rows contiguous in DRAM
        f = a.shape[1]
        return np.ascontiguousarray(
            a.reshape(t, 128, f).transpose(1, 0, 2).reshape(128, t * f))

    xT = {}
    for b in range(B):
        xT[("q", b)] = swz(np.ascontiguousarray(q[b].T), NDT).astype(d_qk)
        xT[("k", b)] = swz(np.ascontiguousarray(k[b].T), NDT).astype(d_qk)
        xT[("v", b)] = swz(np.ascontiguousarray(v[b].T), NDT).astype(d_v)
    wT = {}
    for g in range(HG):
        sl = slice(g * FEAT, (g + 1) * FEAT)
        wk_s = swz(np.ascontiguousarray(Wk[sl, :].T), NDT).astype(d_qk)
        wq_s = swz(np.ascontiguousarray(Wq[sl, :].T), NDT).astype(d_qk)
        bk_s = np.ascontiguousarray(
            bk[sl].astype(np.float32).reshape(2, 128).T).view(np.uint16)
        bq_s = np.ascontiguousarray(
            bq[sl].astype(np.float32).reshape(2, 128).T).view(np.uint16)
        wv_s = swz(np.ascontiguousarray(Wv[sl, :].T), NDT).astype(d_v)
        wo_s = swz(np.ascontiguousarray(Wo[:, sl].T), 2).astype(d_o)
        wT[("kb", g)] = np.ascontiguousarray(np.concatenate(
            [wk_s.view(np.uint16), bk_s], axis=1).view(d_qk))
        wT[("qb", g)] = np.ascontiguousarray(np.concatenate(
            [wq_s.view(np.uint16), bq_s], axis=1).view(d_qk))
        wT[("vo", g)] = np.ascontiguousarray(
            np.concatenate([wv_s, wo_s], axis=1))

    in_maps = []
    for c in range(N_CORES):
        b, g = divmod(c, HG)
        in_maps.append({
            "xqT": xT[("q", b)], "xkT": xT[("k", b)], "xvT": xT[("v", b)],
            "wkbT": wT[("kb", g)], "wqbT": wT[("qb", g)],
            "wvoT": wT[("vo", g)],
        })

    kwargs = {}
    if _trace:
        _install_profile_shim()
        kwargs = dict(trace=True, trace_cores=list(range(N_CORES)))
    res = bass_utils.run_bass_kernel_spmd(
        nc, in_maps, core_ids=list(range(N_CORES)), **kwargs)
    _cache["last_results"] = res

    final_bias = (Wo @ bv + bo).astype(np.float32)  # attn rows sum to 1
    out = np.empty((B, S, D), np.float32)
    for b in range(B):
        acc = res.results[b * HG]["partialT"].astype(np.float32)
        for g in range(1, HG):
            acc += res.results[b * HG + g]["partialT"].astype(np.float32)
        # [qt, p, jt, s] -> [S, D]:  d = jt*128+p, q = qt*512+s
        out[b] = acc.transpose(0, 3, 2, 1).reshape(S, D) + final_bias
    return out


def _install_profile_shim():
    """Provide antenv.axon_hooks so trace=True works under axon."""
    import sys
    import types

    import antenv

    if "antenv.axon_hooks" in sys.modules:
        return
    mod = types.ModuleType("antenv.axon_hooks")
    mod._hook = None
    mod.set_axon_ntff_profile_hook = lambda h: setattr(mod, "_hook", h)
    mod.get_axon_ntff_profile_hook = lambda: mod._hook
    sys.modules["antenv.axon_hooks"] = mod
    antenv.axon_hooks = mod
    try:
        from trn_agent_boot.trn_boot import _ntff_profile_via_ctypes
        mod.set_axon_ntff_profile_hook(
            _ntff_profile_via_ctypes("/opt/axon/libaxon_pjrt.so"))
    except Exception:
        pass


# revision 22
# speedup vs baseline: 1.0656x; 1.0007x over previous
"""MultiHeadAttention Trainium2 kernel (8 NeuronCores, Bass/Tile).

Problem: B=2, S=2048, D=1024, H=16, DK=64 fp32 MHA (torch-Linear style
projections, softmax attention, output projection).

Sharding: core c = (batch b = c//4, head-group g = c%4); each core handles
4 heads of one batch, entirely in a transposed layout (features on
partitions, sequence on the free axis):
  qhT/khT  = (W_g x^T + b)       [2 pairs x 128, 2048]
  vh       = x_v Wv_g^T          [2048, 4x65] (ones col -> row sums)
  scoresT  = khT^T qhT           per (pair, ktile, qtile) -> PSUM
  expT     = exp(scoresT/8)      ACT -> bf16
  rawT     = vh_aug^T expT       PV matmul; row 64 = softmax denominator
  outT     = rawT[0:64] * (1/rawT[64])
  partialT = woT^T outT          [1024, 2048] fp16 -> DRAM
Host: out[b] = sum_g partialT(b,g)^T + (Wo bv + bo).

v2 pipeline notes (v1 measured 257us, PE 75% busy):
- input DMA rings are serviced round-robin, so unordered loads all land
  at ~21us; ring chaining (chain_iter_dep) staggers wk->wq->xk->xq->xv
  so k-proj starts at ~7us.
- warmup matmuls ramp the PE out of its low p-state during the DMA wait.
- e2 exp tiles cycle through the same 32KB pool slots as the (dead by
  then) xk/xq/xv input tiles, giving 2 units of exp/PV pipelining
  without exceeding SBUF.
- partial output written fp16 (halves tail DMA); host sums in fp32.
"""

import numpy as np

B, S, D, H = 2, 2048, 1024, 16
DK = D // H          # 64
N_CORES = 8
HG = H // 4          # 4 head-groups
HL = 4               # heads per core
FEAT = HL * DK       # 256 per-core features
NQT = S // 512       # 4 query tiles
NKT = S // 128       # 16 key tiles
NDT = D // 128       # 8 contraction tiles (d-model)

DT_QK = "fp16"   # x_q/x_k, Wq/Wk, qhT/khT (score operands)
DT_V = "fp16"    # x_v, Wv
DT_PV = "bf16"   # vh_aug, expT
DT_O = "fp16"    # Wo, outT
N_WARMUP = 30    # PE p-state warmup matmuls during initial DMA wait

_cache = {}


def _np_dt(name):
    if name == "fp16":
        return np.float16
    import ml_dtypes
    return ml_dtypes.bfloat16


def _build():
    import concourse.mybir as mybir
    import concourse.tile as tile
    from concourse import bacc

    fp32 = mybir.dt.float32
    dt_qk = getattr(mybir.dt, "float16" if DT_QK == "fp16" else "bfloat16")
    dt_v = getattr(mybir.dt, "float16" if DT_V == "fp16" else "bfloat16")
    dt_pv = getattr(mybir.dt, "float16" if DT_PV == "fp16" else "bfloat16")
    dt_o = getattr(mybir.dt, "float16" if DT_O == "fp16" else "bfloat16")
    dt_out = mybir.dt.float16

    nc = bacc.Bacc("TRN2", target_bir_lowering=False, debug=False,
                   num_devices=N_CORES)

    # all inputs host-swizzled to [128, ...] so each SBUF partition row is
    # ONE contiguous DRAM read. DMA queues round-robin per DESCRIPTOR, so
    # descriptor size acts as priority: x uses 32KB descriptors, weights
    # ~8KB. qk biases ride in the qk weight buffer (fp32 bit-packed into 4
    # trailing fp16 columns, bitcast on device) to avoid tiny descriptors.
    xqT = nc.dram_tensor("xqT", [128, NDT * S], dt_qk,
                         kind="ExternalInput").ap()
    xkT = nc.dram_tensor("xkT", [128, NDT * S], dt_qk,
                         kind="ExternalInput").ap()
    xvT = nc.dram_tensor("xvT", [128, NDT * S], dt_v,
                         kind="ExternalInput").ap()
    wkbT = nc.dram_tensor("wkbT", [128, NDT * FEAT + 4], dt_qk,
                          kind="ExternalInput").ap()
    wqbT = nc.dram_tensor("wqbT", [128, NDT * FEAT + 4], dt_qk,
                          kind="ExternalInput").ap()
    wvoT = nc.dram_tensor("wvoT", [128, NDT * FEAT + 2 * D], dt_v,
                          kind="ExternalInput").ap()
    # output layout [qt, p, jt, s]: each partition row is one contiguous
    # 8KB write (fewer, bigger DMA descriptors); host re-transposes
    out_d = nc.dram_tensor("partialT", [NQT, 128, NDT, 512], dt_out,
                           kind="ExternalOutput").ap()

    xq_r = xqT.rearrange("p (t s) -> p t s", t=NDT)
    xk_r = xkT.rearrange("p (t s) -> p t s", t=NDT)
    xv_r = xvT.rearrange("p (t s) -> p t s", t=NDT)

    with tile.TileContext(nc) as tc:
        def chain(inst, key):
            # stagger DMA ring groups: rings within a group run in parallel
            # (full HBM bw); later groups start only after the prior group's
            # lane finishes, so early tensors land first.
            try:
                tc.chain_iter_dep(key, inst)
            except Exception:
                pass

        with (
            tc.tile_pool(name="win", bufs=1) as win,
            tc.tile_pool(name="big", bufs=4) as big,
            tc.tile_pool(name="proj", bufs=1) as proj,
            tc.tile_pool(name="pout", bufs=1) as pout,
            tc.tile_pool(name="pnrm", bufs=2) as pnrm,
            tc.tile_pool(name="pp", bufs=2, space="PSUM") as pp,
            tc.tile_pool(name="ps2", bufs=2, space="PSUM") as ps2,
            tc.tile_pool(name="pspv", bufs=2, space="PSUM") as pspv,
        ):
            wdum0 = win.tile([128, 512], dt_qk, tag="wdum")
            junk = win.tile([128, 512], dt_qk, tag="junk")
            nc.vector.memset(wdum0[:], 0.0)
            # ---- DMA: one big-descriptor ring per tensor; 3-hop chain
            # xk -> xq -> xv -> wvo so earlier-needed tensors get the full
            # link. wqk (weights+biases) rides unchained beside xk. ----
            wkb = win.tile([128, NDT * FEAT + 4], dt_qk, tag="wkb")
            wqb = win.tile([128, NDT * FEAT + 4], dt_qk, tag="wqb")
            wvo = win.tile([128, NDT * FEAT + 2 * D], dt_v, tag="wvo")

            xk3 = big.tile([128, NDT, S], dt_qk, tag="big")
            xq3 = big.tile([128, NDT, S], dt_qk, tag="big")
            xv3 = big.tile([128, NDT, S], dt_v, tag="big")

            chain(nc.sync.dma_start(wkb[:], wkbT), "l0")
            nc.scalar.activation(junk[0:1, 0:1], wdum0[0:1, 0:1],
                                 mybir.ActivationFunctionType.Exp, scale=1.0)
            for t0 in (0, 4):
                chain(nc.sync.dma_start(xk3[:, t0:t0 + 4, :],
                                        xk_r[:, t0:t0 + 4, :]), "l0")
            chain(nc.sync.dma_start(wqb[:], wqbT), "l0")
            for x3, xr in ((xq3, xq_r), (xv3, xv_r)):
                for t0 in (0, 4):
                    chain(nc.sync.dma_start(x3[:, t0:t0 + 4, :],
                                            xr[:, t0:t0 + 4, :]), "l0")
            chain(nc.sync.dma_start(wvo[:], wvoT), "l0")

            wk3 = wkb[:, 0:NDT * FEAT].rearrange("p (t f) -> p t f", t=NDT)
            wq3 = wqb[:, 0:NDT * FEAT].rearrange("p (t f) -> p t f", t=NDT)
            bk3 = wkb[:, NDT * FEAT:NDT * FEAT + 4].bitcast(fp32)
            bq3 = wqb[:, NDT * FEAT:NDT * FEAT + 4].bitcast(fp32)
            wv3 = wvo[:, 0:NDT * FEAT].rearrange("p (t f) -> p t f", t=NDT)
            wo3 = wvo[:, NDT * FEAT:].rearrange("p (t j) -> p t j", t=2)

            # ---- persistent intermediates ----
            qh3 = proj.tile([128, 2, S], dt_qk, tag="qh")   # pair-packed
            kh3 = proj.tile([128, 2, S], dt_qk, tag="kh")
            vha = proj.tile([128, NKT, HL, DK + 1], dt_pv, tag="vha")
            ot3 = proj.tile([128, 2, S], dt_o, tag="outT")
            nc.gpsimd.memset(vha[:, :, :, DK], 1.0)  # ones col -> denominators

            # ---- PE p-state warmup while the first DMAs land ----
            wdum = wdum0
            wu = pp.tile([128, 512], fp32, tag="acc")
            for i in range(N_WARMUP):
                nc.tensor.matmul(wu[:], wdum[:, 0:128], wdum[:],
                                 start=(i == 0), stop=(i == N_WARMUP - 1))
            nc.vector.tensor_copy(junk[:], wu[:])

            # ---- projections: 2 psum accumulators per pass, kt-interleaved
            # so matmuls chase the chunked x DMAs ----
            def qk_pass(x3, w3, b3, dst, m, nn):
                accs = [pp.tile([128, 512], fp32, tag="acc", name=f"acc{n}")
                        for n in nn]
                for kt in range(NDT):
                    for a, n in zip(accs, nn):
                        nc.tensor.matmul(
                            a[:], w3[:, kt, m * 128:(m + 1) * 128],
                            x3[:, kt, n * 512:(n + 1) * 512],
                            start=(kt == 0), stop=(kt == NDT - 1))
                for a, n in zip(accs, nn):
                    nc.vector.tensor_scalar_add(
                        dst[:, m, n * 512:(n + 1) * 512], a[:],
                        b3[:, m:m + 1])

            def v_quantum(st):
                ps = pp.tile([128, 512], fp32, tag="acc", name="vacc")
                for kt in range(NDT):
                    nc.tensor.matmul(
                        ps[:, 0:256], xv3[:, kt, st * 128:(st + 1) * 128],
                        wv3[:, kt, :],
                        start=(kt == 0), stop=(kt == NDT - 1))
                nc.vector.tensor_copy(vha[:, st, :, 0:DK], ps[:, 0:256])

            def pv_quantum(state, qt, hp, e2u, kt):
                if "a" not in state:
                    state["a"] = pspv.tile([DK + 1, 512], fp32, tag="pv",
                                           name="pva")
                    state["b"] = pspv.tile([DK + 1, 512], fp32, tag="pv",
                                           name="pvb")
                nc.tensor.matmul(
                    state["a"][:], vha[:, kt, 2 * hp, :], e2u[:, kt, 0:512],
                    start=(kt == 0), stop=(kt == NKT - 1))
                nc.tensor.matmul(
                    state["b"][:], vha[:, kt, 2 * hp + 1, :],
                    e2u[:, kt, 512:1024],
                    start=(kt == 0), stop=(kt == NKT - 1))

            def norm(state, qt, hp):
                # whole-accumulator copy frees the PSUM bank early; custom
                # DVE recip needs a base-partition-0 SBUF input (srow)
                for pv, half in ((state["a"], 0), (state["b"], 1)):
                    pvs = pnrm.tile([DK + 1, 512], fp32, tag="pvs")
                    nc.vector.tensor_copy(pvs[:], pv[:])
                    srow = pnrm.tile([1, 512], fp32, tag="srow")
                    nc.vector.tensor_copy(srow[:], pvs[DK:DK + 1, :])
                    inv = pnrm.tile([1, 512], fp32, tag="inv")
                    nc.vector.reciprocal_approx_fast(inv[:], srow[:])
                    invb = pnrm.tile([64, 512], fp32, tag="invb")
                    nc.gpsimd.partition_broadcast(invb[:], inv[:])
                    nc.vector.tensor_tensor(
                        ot3[half * 64:(half + 1) * 64, hp,
                            qt * 512:(qt + 1) * 512],
                        pvs[0:DK, :], invb[:], mybir.AluOpType.mult)

            def oproj_quantum(pstate, qt, jt):
                if "po" not in pstate:
                    pstate["po"] = pout.tile([128, NDT, 512], dt_out,
                                             tag="po", bufs=1, name="po")
                ps = pp.tile([128, 512], fp32, tag="acc", name="oacc")
                for m in range(2):
                    nc.tensor.matmul(
                        ps[:], wo3[:, m, jt * 128:(jt + 1) * 128],
                        ot3[:, m, qt * 512:(qt + 1) * 512],
                        start=(m == 0), stop=(m == 1))
                nc.vector.tensor_copy(pstate["po"][:, jt, :], ps[:])
                if jt == NDT - 1:
                    nc.sync.dma_start(out_d[qt], pstate["po"][:])

            def e2tile(name):
                return big.tile([128, NKT, 1024], dt_pv, tag="big", name=name)

            # ---- filler queue: PE work that rides in the slack of the
            # exp-paced scores stream (ACT needs ~1088ns/kt, scores only
            # ~426ns of PE) so the exp stream never starves ----
            fillers = []      # list of (cost_ns, thunk)
            fq = {"i": 0, "budget": 0.0}

            def drain_fillers():
                while fq["i"] < len(fillers):
                    fillers[fq["i"]][1]()
                    fq["i"] += 1

            SLACK_NS = 680    # filler budget added per exp-paced kt step

            def s_unit(qt, hp, e2u, fill=True):
                for kt in range(NKT):
                    s2 = ps2.tile([128, 1024], fp32, tag="s2")
                    nc.tensor.matmul(
                        s2[:, 0:512],
                        kh3[0:64, hp, kt * 128:(kt + 1) * 128],
                        qh3[0:64, hp, qt * 512:(qt + 1) * 512],
                        start=True, stop=True)
                    nc.tensor.matmul(
                        s2[:, 512:1024],
                        kh3[64:128, hp, kt * 128:(kt + 1) * 128],
                        qh3[64:128, hp, qt * 512:(qt + 1) * 512],
                        start=True, stop=True)
                    nc.scalar.activation(
                        e2u[:, kt, :], s2[:],
                        mybir.ActivationFunctionType.Exp, scale=0.125)
                    if not fill:
                        continue
                    fq["budget"] += SLACK_NS
                    while (fq["i"] < len(fillers)
                           and fillers[fq["i"]][0] <= fq["budget"]):
                        cost, fn = fillers[fq["i"]]
                        fq["budget"] -= cost
                        fn()
                        fq["i"] += 1

            # ---- emission: K, q-heads, then the exp-paced score stream
            # with everything else as fillers. Queue order respects the
            # e2-slot reuse deps (all xq readers before S(1,0)'s exps emit,
            # all xv readers before S(1,1)'s). ----
            for m in range(2):                       # k-proj (all pairs)
                for nn in ((0, 1), (2, 3)):
                    qk_pass(xk3, wk3, bk3, kh3, m, nn)
            qk_pass(xq3, wq3, bq3, qh3, 0, (0,))     # q-proj heads only
            qk_pass(xq3, wq3, bq3, qh3, 1, (0,))

            for m in range(2):                       # q remainder -> fillers
                for n in (1, 2, 3):
                    fillers.append(
                        (1800, lambda m=m, n=n: qk_pass(
                            xq3, wq3, bq3, qh3, m, (n,))))

            e00 = e2tile("e00")
            s_unit(0, 0, e00, fill=False)            # ACT starts here
            e01 = e2tile("e01")
            s_unit(0, 1, e01)
            # all xq readers are queued; force them out before S(1,0) emits
            # exp writes into xq3's recycled slot
            drain_fillers()
            for st in range(NKT):                    # v-proj -> fillers
                fillers.append((900, lambda st=st: v_quantum(st)))
            e10 = e2tile("e10")
            s_unit(1, 0, e10)
            drain_fillers()                          # xv readers, see above
            pv_states = {}
            for u, (uq, uh, eu) in enumerate(((0, 0, e00), (0, 1, e01))):
                st_ = pv_states[(uq, uh)] = {}
                for kt in range(NKT):
                    fillers.append(
                        (440, lambda s=st_, q=uq, h=uh, e=eu, k=kt:
                         pv_quantum(s, q, h, e, k)))
                fillers.append(
                    (0, lambda s=st_, q=uq, h=uh: norm(s, q, h)))
            e11 = e2tile("e11")
            s_unit(1, 1, e11)

            prev = {(1, 0): e10, (1, 1): e11}
            for qt in range(1, NQT):
                for hp in range(2):
                    if qt == NQT - 1 and hp == 1:
                        continue          # last unit's PV runs in the tail
                    st_ = pv_states[(qt, hp)] = {}
                    eu = prev[(qt, hp)]
                    for kt in range(NKT):
                        fillers.append(
                            (440, lambda s=st_, q=qt, h=hp, e=eu, k=kt:
                             pv_quantum(s, q, h, e, k)))
                    fillers.append(
                        (0, lambda s=st_, q=qt, h=hp: norm(s, q, h)))
                    if hp == 1:
                        pstate = {}
                        for jt in range(NDT):
                            fillers.append(
                                (480, lambda p=pstate, q=qt - 1, j=jt:
                                 oproj_quantum(p, q, j)))
                if qt == NQT - 1:
                        pstate = {}
                        for jt in range(NDT):
                            fillers.append(
                                (480, lambda p=pstate, q=qt - 1, j=jt:
                                 oproj_quantum(p, q, j)))
                # emit next qt's score units
                if qt < NQT - 1:
                    for hp in range(2):
                        e_n = e2tile(f"e{qt + 1}{hp}")
                        prev[(qt + 1, hp)] = e_n
                        s_unit(qt + 1, hp, e_n)

            # tail: leftover fillers, last PV tracking the last exps, then
            # keeper matmuls bridge the final normalize so oproj(3) stays at
            # full clock
            drain_fillers()
            st_ = pv_states[(3, 1)] = {}
            for kt in range(NKT):
                pv_quantum(st_, 3, 1, prev[(3, 1)], kt)
            wu2 = pp.tile([128, 512], fp32, tag="acc", name="wu2")
            for i in range(10):
                nc.tensor.matmul(wu2[:], wdum[:, 0:128], wdum[:],
                                 start=(i == 0), stop=(i == 9))
            norm(st_, 3, 1)
            nc.vector.tensor_copy(junk[:], wu2[:])
            pstate = {}
            for jt in range(NDT):
                oproj_quantum(pstate, 3, jt)

    nc.compile()
    return nc


def kernel(q, k, v, Wq, bq, Wk, bk, Wv, bv, Wo, bo, _trace=False):
    from concourse import bass_utils

    if "nc" not in _cache:
        _cache["nc"] = _build()
    nc = _cache["nc"]

    q = np.asarray(q, np.float32)
    k = np.asarray(k, np.float32)
    v = np.asarray(v, np.float32)
    Wq = np.asarray(Wq, np.float32)
    Wk = np.asarray(Wk, np.float32)
    Wv = np.asarray(Wv, np.float32)
    Wo = np.asarray(Wo, np.float32)
    bq = np.asarray(bq, np.float32)
    bk = np.asarray(bk, np.float32)
    bv = np.asarray(bv, np.float32)
    bo = np.asarray(bo, np.float32)

    d_qk, d_v, d_o = _np_dt(DT_QK), _np_dt(DT_V), _np_dt(DT_O)

    def swz(a, t):     # [t*128, f] -> [128, t*f], rows contiguous in DRAM
        f = a.shape[1]
        return np.ascontiguousarray(
            a.reshape(t, 128, f).transpose(1, 0, 2).reshape(128, t * f))

    xT = {}
    for b in range(B):
        xT[("q", b)] = swz(np.ascontiguousarray(q[b].T), NDT).astype(d_qk)
        xT[("k", b)] = swz(np.ascontiguousarray(k[b].T), NDT).astype(d_qk)
        xT[("v", b)] = swz(np.ascontiguousarray(v[b].T), NDT).astype(d_v)
    wT = {}
    for g in range(HG):
        sl = slice(g * FEAT, (g + 1) * FEAT)
        wk_s = swz(np.ascontiguousarray(Wk[sl, :].T), NDT).astype(d_qk)
        wq_s = swz(np.ascontiguousarray(Wq[sl, :].T), NDT).astype(d_qk)
        bk_s = np.ascontiguousarray(
            bk[sl].astype(np.float32).reshape(2, 128).T).view(np.uint16)
        bq_s = np.ascontiguousarray(
            bq[sl].astype(np.float32).reshape(2, 128).T).view(np.uint16)
        wv_s = swz(np.ascontiguousarray(Wv[sl, :].T), NDT).astype(d_v)
        wo_s = swz(np.ascontiguousarray(Wo[:, sl].T), 2).astype(d_o)
        wT[("kb", g)] = np.ascontiguousarray(np.concatenate(
            [wk_s.view(np.uint16), bk_s], axis=1).view(d_qk))
        wT[("qb", g)] = np.ascontiguousarray(np.concatenate(
            [wq_s.view(np.uint16), bq_s], axis=1).view(d_qk))
        wT[("vo", g)] = np.ascontiguousarray(
            np.concatenate([wv_s, wo_s], axis=1))

    in_maps = []
    for c in range(N_CORES):
        b, g = divmod(c, HG)
        in_maps.append({
            "xqT": xT[("q", b)], "xkT": xT[("k", b)], "xvT": xT[("v", b)],
            "wkbT": wT[("kb", g)], "wqbT": wT[("qb", g)],
            "wvoT": wT[("vo", g)],
        })

    kwargs = {}
    if _trace:
        _install_profile_shim()
        kwargs = dict(trace=True, trace_cores=list(range(N_CORES)))
    res = bass_utils.run_bass_kernel_spmd(
        nc, in_maps, core_ids=list(range(N_CORES)), **kwargs)
    _cache["last_results"] = res

    final_bias = (Wo @ bv + bo).astype(np.float32)  # attn rows sum to 1
    out = np.empty((B, S, D), np.float32)
    for b in range(B):
        acc = res.results[b * HG]["partialT"].astype(np.float32)
        for g in range(1, HG):
            acc += res.results[b * HG + g]["partialT"].astype(np.float32)
        # [qt, p, jt, s] -> [S, D]:  d = jt*128+p, q = qt*512+s
        out[b] = acc.transpose(0, 3, 2, 1).reshape(S, D) + final_bias
    return out


def _install_profile_shim():
    """Provide antenv.axon_hooks so trace=True works under axon."""
    import sys
    import types

    import antenv

    if "antenv.axon_hooks" in sys.modules:
        return
    mod = types.ModuleType("antenv.axon_hooks")
    mod._hook = None
    mod.set_axon_ntff_profile_hook = lambda h: setattr(mod, "_hook", h)
    mod.get_axon_ntff_profile_hook = lambda: mod._hook
    sys.modules["antenv.axon_hooks"] = mod
    antenv.axon_hooks = mod
    try:
        from trn_agent_boot.trn_boot import _ntff_profile_via_ctypes
        mod.set_axon_ntff_profile_hook(
            _ntff_profile_via_ctypes("/opt/axon/libaxon_pjrt.so"))
    except Exception:
        pass


# revision 23
# speedup vs baseline: 1.0658x; 1.0002x over previous
"""MultiHeadAttention Trainium2 kernel (8 NeuronCores, Bass/Tile).

Problem: B=2, S=2048, D=1024, H=16, DK=64 fp32 MHA (torch-Linear style
projections, softmax attention, output projection).

Sharding: core c = (batch b = c//4, head-group g = c%4); each core handles
4 heads of one batch, entirely in a transposed layout (features on
partitions, sequence on the free axis):
  qhT/khT  = (W_g x^T + b)       [2 pairs x 128, 2048]
  vh       = x_v Wv_g^T          [2048, 4x65] (ones col -> row sums)
  scoresT  = khT^T qhT           per (pair, ktile, qtile) -> PSUM
  expT     = exp(scoresT/8)      ACT -> bf16
  rawT     = vh_aug^T expT       PV matmul; row 64 = softmax denominator
  outT     = rawT[0:64] * (1/rawT[64])
  partialT = woT^T outT          [1024, 2048] fp16 -> DRAM
Host: out[b] = sum_g partialT(b,g)^T + (Wo bv + bo).

v2 pipeline notes (v1 measured 257us, PE 75% busy):
- input DMA rings are serviced round-robin, so unordered loads all land
  at ~21us; ring chaining (chain_iter_dep) staggers wk->wq->xk->xq->xv
  so k-proj starts at ~7us.
- warmup matmuls ramp the PE out of its low p-state during the DMA wait.
- e2 exp tiles cycle through the same 32KB pool slots as the (dead by
  then) xk/xq/xv input tiles, giving 2 units of exp/PV pipelining
  without exceeding SBUF.
- partial output written fp16 (halves tail DMA); host sums in fp32.
"""

import numpy as np

B, S, D, H = 2, 2048, 1024, 16
DK = D // H          # 64
N_CORES = 8
HG = H // 4          # 4 head-groups
HL = 4               # heads per core
FEAT = HL * DK       # 256 per-core features
NQT = S // 512       # 4 query tiles
NKT = S // 128       # 16 key tiles
NDT = D // 128       # 8 contraction tiles (d-model)

DT_QK = "fp16"   # x_q/x_k, Wq/Wk, qhT/khT (score operands)
DT_V = "fp16"    # x_v, Wv
DT_PV = "bf16"   # vh_aug, expT
DT_O = "fp16"    # Wo, outT
N_WARMUP = 40    # PE p-state warmup matmuls during initial DMA wait

_cache = {}


def _np_dt(name):
    if name == "fp16":
        return np.float16
    import ml_dtypes
    return ml_dtypes.bfloat16


def _build():
    import concourse.mybir as mybir
    import concourse.tile as tile
    from concourse import bacc

    fp32 = mybir.dt.float32
    dt_qk = getattr(mybir.dt, "float16" if DT_QK == "fp16" else "bfloat16")
    dt_v = getattr(mybir.dt, "float16" if DT_V == "fp16" else "bfloat16")
    dt_pv = getattr(mybir.dt, "float16" if DT_PV == "fp16" else "bfloat16")
    dt_o = getattr(mybir.dt, "float16" if DT_O == "fp16" else "bfloat16")
    dt_out = mybir.dt.float16

    nc = bacc.Bacc("TRN2", target_bir_lowering=False, debug=False,
                   num_devices=N_CORES)

    # all inputs host-swizzled to [128, ...] so each SBUF partition row is
    # ONE contiguous DRAM read. DMA queues round-robin per DESCRIPTOR, so
    # descriptor size acts as priority: x uses 32KB descriptors, weights
    # ~8KB. qk biases ride in the qk weight buffer (fp32 bit-packed into 4
    # trailing fp16 columns, bitcast on device) to avoid tiny descriptors.
    xqT = nc.dram_tensor("xqT", [128, NDT * S], dt_qk,
                         kind="ExternalInput").ap()
    xkT = nc.dram_tensor("xkT", [128, NDT * S], dt_qk,
                         kind="ExternalInput").ap()
    xvT = nc.dram_tensor("xvT", [128, NDT * S], dt_v,
                         kind="ExternalInput").ap()
    wkbT = nc.dram_tensor("wkbT", [128, NDT * FEAT + 4], dt_qk,
                          kind="ExternalInput").ap()
    wqbT = nc.dram_tensor("wqbT", [128, NDT * FEAT + 4], dt_qk,
                          kind="ExternalInput").ap()
    wvoT = nc.dram_tensor("wvoT", [128, NDT * FEAT + 2 * D], dt_v,
                          kind="ExternalInput").ap()
    # output layout [qt, p, jt, s]: each partition row is one contiguous
    # 8KB write (fewer, bigger DMA descriptors); host re-transposes
    out_d = nc.dram_tensor("partialT", [NQT, 128, NDT, 512], dt_out,
                           kind="ExternalOutput").ap()

    xq_r = xqT.rearrange("p (t s) -> p t s", t=NDT)
    xk_r = xkT.rearrange("p (t s) -> p t s", t=NDT)
    xv_r = xvT.rearrange("p (t s) -> p t s", t=NDT)

    with tile.TileContext(nc) as tc:
        def chain(inst, key):
            # stagger DMA ring groups: rings within a group run in parallel
            # (full HBM bw); later groups start only after the prior group's
            # lane finishes, so early tensors land first.
            try:
                tc.chain_iter_dep(key, inst)
            except Exception:
                pass

        with (
            tc.tile_pool(name="win", bufs=1) as win,
            tc.tile_pool(name="big", bufs=4) as big,
            tc.tile_pool(name="proj", bufs=1) as proj,
            tc.tile_pool(name="pout", bufs=1) as pout,
            tc.tile_pool(name="pnrm", bufs=2) as pnrm,
            tc.tile_pool(name="pp", bufs=2, space="PSUM") as pp,
            tc.tile_pool(name="ps2", bufs=2, space="PSUM") as ps2,
            tc.tile_pool(name="pspv", bufs=2, space="PSUM") as pspv,
        ):
            wdum0 = win.tile([128, 512], dt_qk, tag="wdum")
            junk = win.tile([128, 512], dt_qk, tag="junk")
            nc.vector.memset(wdum0[:], 0.0)
            # ---- DMA: one big-descriptor ring per tensor; 3-hop chain
            # xk -> xq -> xv -> wvo so earlier-needed tensors get the full
            # link. wqk (weights+biases) rides unchained beside xk. ----
            wkb = win.tile([128, NDT * FEAT + 4], dt_qk, tag="wkb")
            wqb = win.tile([128, NDT * FEAT + 4], dt_qk, tag="wqb")
            wvo = win.tile([128, NDT * FEAT + 2 * D], dt_v, tag="wvo")

            xk3 = big.tile([128, NDT, S], dt_qk, tag="big")
            xq3 = big.tile([128, NDT, S], dt_qk, tag="big")
            xv3 = big.tile([128, NDT, S], dt_v, tag="big")

            chain(nc.sync.dma_start(wkb[:], wkbT), "l0")
            nc.scalar.activation(junk[0:1, 0:1], wdum0[0:1, 0:1],
                                 mybir.ActivationFunctionType.Exp, scale=1.0)
            for t0 in (0, 4):
                chain(nc.sync.dma_start(xk3[:, t0:t0 + 4, :],
                                        xk_r[:, t0:t0 + 4, :]), "l0")
            chain(nc.sync.dma_start(wqb[:], wqbT), "l0")
            for x3, xr in ((xq3, xq_r), (xv3, xv_r)):
                for t0 in (0, 4):
                    chain(nc.sync.dma_start(x3[:, t0:t0 + 4, :],
                                            xr[:, t0:t0 + 4, :]), "l0")
            chain(nc.sync.dma_start(wvo[:], wvoT), "l0")

            wk3 = wkb[:, 0:NDT * FEAT].rearrange("p (t f) -> p t f", t=NDT)
            wq3 = wqb[:, 0:NDT * FEAT].rearrange("p (t f) -> p t f", t=NDT)
            bk3 = wkb[:, NDT * FEAT:NDT * FEAT + 4].bitcast(fp32)
            bq3 = wqb[:, NDT * FEAT:NDT * FEAT + 4].bitcast(fp32)
            wv3 = wvo[:, 0:NDT * FEAT].rearrange("p (t f) -> p t f", t=NDT)
            wo3 = wvo[:, NDT * FEAT:].rearrange("p (t j) -> p t j", t=2)

            # ---- persistent intermediates ----
            qh3 = proj.tile([128, 2, S], dt_qk, tag="qh")   # pair-packed
            kh3 = proj.tile([128, 2, S], dt_qk, tag="kh")
            vha = proj.tile([128, NKT, HL, DK + 1], dt_pv, tag="vha")
            ot3 = proj.tile([128, 2, S], dt_o, tag="outT")
            nc.gpsimd.memset(vha[:, :, :, DK], 1.0)  # ones col -> denominators

            # ---- PE p-state warmup while the first DMAs land ----
            wdum = wdum0
            wu = pp.tile([128, 512], fp32, tag="acc")
            for i in range(N_WARMUP):
                nc.tensor.matmul(wu[:], wdum[:, 0:128], wdum[:],
                                 start=(i == 0), stop=(i == N_WARMUP - 1))
            nc.vector.tensor_copy(junk[:], wu[:])

            # ---- projections: 2 psum accumulators per pass, kt-interleaved
            # so matmuls chase the chunked x DMAs ----
            def qk_pass(x3, w3, b3, dst, m, nn):
                accs = [pp.tile([128, 512], fp32, tag="acc", name=f"acc{n}")
                        for n in nn]
                for kt in range(NDT):
                    for a, n in zip(accs, nn):
                        nc.tensor.matmul(
                            a[:], w3[:, kt, m * 128:(m + 1) * 128],
                            x3[:, kt, n * 512:(n + 1) * 512],
                            start=(kt == 0), stop=(kt == NDT - 1))
                for a, n in zip(accs, nn):
                    nc.vector.tensor_scalar_add(
                        dst[:, m, n * 512:(n + 1) * 512], a[:],
                        b3[:, m:m + 1])

            def v_quantum(st):
                ps = pp.tile([128, 512], fp32, tag="acc", name="vacc")
                for kt in range(NDT):
                    nc.tensor.matmul(
                        ps[:, 0:256], xv3[:, kt, st * 128:(st + 1) * 128],
                        wv3[:, kt, :],
                        start=(kt == 0), stop=(kt == NDT - 1))
                nc.vector.tensor_copy(vha[:, st, :, 0:DK], ps[:, 0:256])

            def pv_quantum(state, qt, hp, e2u, kt):
                if "a" not in state:
                    state["a"] = pspv.tile([DK + 1, 512], fp32, tag="pv",
                                           name="pva")
                    state["b"] = pspv.tile([DK + 1, 512], fp32, tag="pv",
                                           name="pvb")
                nc.tensor.matmul(
                    state["a"][:], vha[:, kt, 2 * hp, :], e2u[:, kt, 0:512],
                    start=(kt == 0), stop=(kt == NKT - 1))
                nc.tensor.matmul(
                    state["b"][:], vha[:, kt, 2 * hp + 1, :],
                    e2u[:, kt, 512:1024],
                    start=(kt == 0), stop=(kt == NKT - 1))

            def norm(state, qt, hp):
                # whole-accumulator copy frees the PSUM bank early; custom
                # DVE recip needs a base-partition-0 SBUF input (srow)
                for pv, half in ((state["a"], 0), (state["b"], 1)):
                    pvs = pnrm.tile([DK + 1, 512], fp32, tag="pvs")
                    nc.vector.tensor_copy(pvs[:], pv[:])
                    srow = pnrm.tile([1, 512], fp32, tag="srow")
                    nc.vector.tensor_copy(srow[:], pvs[DK:DK + 1, :])
                    inv = pnrm.tile([1, 512], fp32, tag="inv")
                    nc.vector.reciprocal_approx_fast(inv[:], srow[:])
                    invb = pnrm.tile([64, 512], fp32, tag="invb")
                    nc.gpsimd.partition_broadcast(invb[:], inv[:])
                    nc.vector.tensor_tensor(
                        ot3[half * 64:(half + 1) * 64, hp,
                            qt * 512:(qt + 1) * 512],
                        pvs[0:DK, :], invb[:], mybir.AluOpType.mult)

            def oproj_quantum(pstate, qt, jt):
                if "po" not in pstate:
                    pstate["po"] = pout.tile([128, NDT, 512], dt_out,
                                             tag="po", bufs=1, name="po")
                ps = pp.tile([128, 512], fp32, tag="acc", name="oacc")
                for m in range(2):
                    nc.tensor.matmul(
                        ps[:], wo3[:, m, jt * 128:(jt + 1) * 128],
                        ot3[:, m, qt * 512:(qt + 1) * 512],
                        start=(m == 0), stop=(m == 1))
                nc.vector.tensor_copy(pstate["po"][:, jt, :], ps[:])
                if qt == NQT - 1:
                    # last tile: per-jt DMA starts the final drain earlier
                    nc.sync.dma_start(out_d[qt, :, jt:jt + 1, :],
                                      pstate["po"][:, jt:jt + 1, :])
                elif jt == NDT - 1:
                    nc.sync.dma_start(out_d[qt], pstate["po"][:])

            def e2tile(name):
                return big.tile([128, NKT, 1024], dt_pv, tag="big", name=name)

            # ---- filler queue: PE work that rides in the slack of the
            # exp-paced scores stream (ACT needs ~1088ns/kt, scores only
            # ~426ns of PE) so the exp stream never starves ----
            fillers = []      # list of (cost_ns, thunk)
            fq = {"i": 0, "budget": 0.0}

            def drain_fillers():
                while fq["i"] < len(fillers):
                    fillers[fq["i"]][1]()
                    fq["i"] += 1

            SLACK_NS = 680    # filler budget added per exp-paced kt step

            def s_unit(qt, hp, e2u, fill=True):
                for kt in range(NKT):
                    s2 = ps2.tile([128, 1024], fp32, tag="s2")
                    nc.tensor.matmul(
                        s2[:, 0:512],
                        kh3[0:64, hp, kt * 128:(kt + 1) * 128],
                        qh3[0:64, hp, qt * 512:(qt + 1) * 512],
                        start=True, stop=True)
                    nc.tensor.matmul(
                        s2[:, 512:1024],
                        kh3[64:128, hp, kt * 128:(kt + 1) * 128],
                        qh3[64:128, hp, qt * 512:(qt + 1) * 512],
                        start=True, stop=True)
                    nc.scalar.activation(
                        e2u[:, kt, :], s2[:],
                        mybir.ActivationFunctionType.Exp, scale=0.125)
                    if not fill:
                        continue
                    fq["budget"] += SLACK_NS
                    while (fq["i"] < len(fillers)
                           and fillers[fq["i"]][0] <= fq["budget"]):
                        cost, fn = fillers[fq["i"]]
                        fq["budget"] -= cost
                        fn()
                        fq["i"] += 1

            # ---- emission: K, q-heads, then the exp-paced score stream
            # with everything else as fillers. Queue order respects the
            # e2-slot reuse deps (all xq readers before S(1,0)'s exps emit,
            # all xv readers before S(1,1)'s). ----
            for m in range(2):                       # k-proj (all pairs)
                for nn in ((0, 1), (2, 3)):
                    qk_pass(xk3, wk3, bk3, kh3, m, nn)
            qk_pass(xq3, wq3, bq3, qh3, 0, (0,))     # q-proj heads only
            qk_pass(xq3, wq3, bq3, qh3, 1, (0,))

            for m in range(2):                       # q remainder -> fillers
                for n in (1, 2, 3):
                    fillers.append(
                        (1800, lambda m=m, n=n: qk_pass(
                            xq3, wq3, bq3, qh3, m, (n,))))

            e00 = e2tile("e00")
            s_unit(0, 0, e00, fill=False)            # ACT starts here
            e01 = e2tile("e01")
            s_unit(0, 1, e01)
            # all xq readers are queued; force them out before S(1,0) emits
            # exp writes into xq3's recycled slot
            drain_fillers()
            for st in range(NKT):                    # v-proj -> fillers
                fillers.append((900, lambda st=st: v_quantum(st)))
            e10 = e2tile("e10")
            s_unit(1, 0, e10)
            drain_fillers()                          # xv readers, see above
            pv_states = {}
            for u, (uq, uh, eu) in enumerate(((0, 0, e00), (0, 1, e01))):
                st_ = pv_states[(uq, uh)] = {}
                for kt in range(NKT):
                    fillers.append(
                        (440, lambda s=st_, q=uq, h=uh, e=eu, k=kt:
                         pv_quantum(s, q, h, e, k)))
                fillers.append(
                    (0, lambda s=st_, q=uq, h=uh: norm(s, q, h)))
            e11 = e2tile("e11")
            s_unit(1, 1, e11)

            prev = {(1, 0): e10, (1, 1): e11}
            for qt in range(1, NQT):
                for hp in range(2):
                    if qt == NQT - 1 and hp == 1:
                        continue          # last unit's PV runs in the tail
                    st_ = pv_states[(qt, hp)] = {}
                    eu = prev[(qt, hp)]
                    for kt in range(NKT):
                        fillers.append(
                            (440, lambda s=st_, q=qt, h=hp, e=eu, k=kt:
                             pv_quantum(s, q, h, e, k)))
                    fillers.append(
                        (0, lambda s=st_, q=qt, h=hp: norm(s, q, h)))
                    if hp == 1:
                        pstate = {}
                        for jt in range(NDT):
                            fillers.append(
                                (480, lambda p=pstate, q=qt - 1, j=jt:
                                 oproj_quantum(p, q, j)))
                if qt == NQT - 1:
                        pstate = {}
                        for jt in range(NDT):
                            fillers.append(
                                (480, lambda p=pstate, q=qt - 1, j=jt:
                                 oproj_quantum(p, q, j)))
                # emit next qt's score units
                if qt < NQT - 1:
                    for hp in range(2):
                        e_n = e2tile(f"e{qt + 1}{hp}")
                        prev[(qt + 1, hp)] = e_n
                        s_unit(qt + 1, hp, e_n)

            # tail: leftover fillers, last PV tracking the last exps, then
            # keeper matmuls bridge the final normalize so oproj(3) stays at
            # full clock
            drain_fillers()
            st_ = pv_states[(3, 1)] = {}
            for kt in range(NKT):
                pv_quantum(st_, 3, 1, prev[(3, 1)], kt)
            wu2 = pp.tile([128, 512], fp32, tag="acc", name="wu2")
            for i in range(10):
                nc.tensor.matmul(wu2[:], wdum[:, 0:128], wdum[:],
                                 start=(i == 0), stop=(i == 9))
            norm(st_, 3, 1)
            nc.vector.tensor_copy(junk[:], wu2[:])
            pstate = {}
            for jt in range(NDT):
                oproj_quantum(pstate, 3, jt)

    nc.compile()
    return nc


def kernel(q, k, v, Wq, bq, Wk, bk, Wv, bv, Wo, bo, _trace=False):
    from concourse import bass_utils

    if "nc" not in _cache:
        _cache["nc"] = _build()
    nc = _cache["nc"]

    q = np.asarray(q, np.float32)
    k = np.asarray(k, np.float32)
    v = np.asarray(v, np.float32)
    Wq = np.asarray(Wq, np.float32)
    Wk = np.asarray(Wk, np.float32)
    Wv = np.asarray(Wv, np.float32)
    Wo = np.asarray(Wo, np.float32)
    bq = np.asarray(bq, np.float32)
    bk = np.asarray(bk, np.float32)
    bv = np.asarray(bv, np.float32)
    bo = np.asarray(bo, np.float32)

    d_qk, d_v, d_o = _np_dt(DT_QK), _np_dt(DT_V), _np_dt(DT_O)

    def swz(a, t):     # [t*128, f] -> [128, t*f], rows contiguous in DRAM
        f = a.shape[1]
        return np.ascontiguousarray(
            a.reshape(t, 128, f).transpose(1, 0, 2).reshape(128, t * f))

    xT = {}
    for b in range(B):
        xT[("q", b)] = swz(np.ascontiguousarray(q[b].T), NDT).astype(d_qk)
        xT[("k", b)] = swz(np.ascontiguousarray(k[b].T), NDT).astype(d_qk)
        xT[("v", b)] = swz(np.ascontiguousarray(v[b].T), NDT).astype(d_v)
    wT = {}
    for g in range(HG):
        sl = slice(g * FEAT, (g + 1) * FEAT)
        wk_s = swz(np.ascontiguousarray(Wk[sl, :].T), NDT).astype(d_qk)
        wq_s = swz(np.ascontiguousarray(Wq[sl, :].T), NDT).astype(d_qk)
        bk_s = np.ascontiguousarray(
            bk[sl].astype(np.float32).reshape(2, 128).T).view(np.uint16)
        bq_s = np.ascontiguousarray(
            bq[sl].astype(np.float32).reshape(2, 128).T).view(np.uint16)
        wv_s = swz(np.ascontiguousarray(Wv[sl, :].T), NDT).astype(d_v)
        wo_s = swz(np.ascontiguousarray(Wo[:, sl].T), 2).astype(d_o)
        wT[("kb", g)] = np.ascontiguousarray(np.concatenate(
            [wk_s.view(np.uint16), bk_s], axis=1).view(d_qk))
        wT[("qb", g)] = np.ascontiguousarray(np.concatenate(
            [wq_s.view(np.uint16), bq_s], axis=1).view(d_qk))
        wT[("vo", g)] = np.ascontiguousarray(
            np.concatenate([wv_s, wo_s], axis=1))

    in_maps = []
    for c in range(N_CORES):
        b, g = divmod(c, HG)
        in_maps.append({
            "xqT": xT[("q", b)], "xkT": xT[("k", b)], "xvT": xT[("v", b)],
            "wkbT": wT[("kb", g)], "wqbT": wT[("qb", g)],
            "wvoT": wT[("vo", g)],
        })

    kwargs = {}
    if _trace:
        _install_profile_shim()
        kwargs = dict(trace=True, trace_cores=list(range(N_CORES)))
    res = bass_utils.run_bass_kernel_spmd(
        nc, in_maps, core_ids=list(range(N_CORES)), **kwargs)
    _cache["last_results"] = res

    final_bias = (Wo @ bv + bo).astype(np.float32)  # attn rows sum to 1
    out = np.empty((B, S, D), np.float32)
    for b in range(B):
        acc = res.results[b * HG]["partialT"].astype(np.float32)
        for g in range(1, HG):
            acc += res.results[b * HG + g]["partialT"].astype(np.float32)
        # [qt, p, jt, s] -> [S, D]:  d = jt*128+p, q = qt*512+s
        out[b] = acc.transpose(0, 3, 2, 1).reshape(S, D) + final_bias
    return out


def _install_profile_shim():
    """Provide antenv.axon_hooks so trace=True works under axon."""
    import sys
    import types

    import antenv

    if "antenv.axon_hooks" in sys.modules:
        return
    mod = types.ModuleType("antenv.axon_hooks")
    mod._hook = None
    mod.set_axon_ntff_profile_hook = lambda h: setattr(mod, "_hook", h)
    mod.get_axon_ntff_profile_hook = lambda: mod._hook
    sys.modules["antenv.axon_hooks"] = mod
    antenv.axon_hooks = mod
    try:
        from trn_agent_boot.trn_boot import _ntff_profile_via_ctypes
        mod.set_axon_ntff_profile_hook(
            _ntff_profile_via_ctypes("/opt/axon/libaxon_pjrt.so"))
    except Exception:
        pass
